# revision 29
# baseline (speedup 1.0000x reference)
"""Sparse-attention (sliding window 512 + front 256) Trainium2 kernel.

Sequence-sharded across 8 NeuronCores: core c owns queries [512c, 512c+512)
and computes ALL 16 heads for them, including the full output projection, so
per-core output is a disjoint y slice [512, 2048] (no cross-core reduction).

Layout: scores are computed TRANSPOSED (sT[k, q] = kT_tile^T @ qT), so the
exp() output is already the [keys, q] rhs operand the attn@V matmul needs --
no PE transposes and no PSUM->SBUF staging copies. The softmax key-dim sum is
a row-select-vector matmul on the PE; 1/lsum (reciprocal_approx_fast) is
broadcast across partitions with a rank-1 ones matmul and applied to the
attn@V PSUM output by the DVE on its way to SBUF.

Uniform single program across cores; per-core variation is data only:
  - packed key layout (NT=10 tiles): cols 0:256 front, cols 256:1280 band
    positions [qlo-512, qlo+512), zero-x for padding (and for core 0's dead
    front tiles, which its band range covers);
  - 11 sparse 128x128 {0,1} mask blocks (the only blocks not full on every
    core: anti/causal window edges + core-0/1 front edge cases), applied
    multiplicatively to exp(scores) -- front tiles use only the front_ok
    condition, band tiles only the band condition (disjoint), so duplicated
    front/band keys on cores 0-1 never double count;
  - a [128, NT] row-select vector excluding padding/dead rows from lsum.

Phase pipeline: A) k/v/q projections + paired-basis RoPE, chunk-ordered so
compute starts with the first x DMA; B) attention in head pairs (20 score
tiles batched ahead of the lsum/attn@V runs, normalize tail software-
pipelined one pair behind); C) y projection streaming wo in n-chunks,
h-accumulation letting the scheduler overlap C with B's last heads.
"""
import math
import sys

import numpy as np

sys.path.insert(0, "/opt/trn_rl_repo")

import concourse.bass as bass
from concourse import bacc
import concourse.mybir as mybir
import concourse.tile as tile
from concourse.bass_utils import run_bass_kernel_spmd

# Problem constants (hardcoded per contract)
S = 4096
D = 2048
NH = 16
NKV = 4
NREP = NH // NKV
DQK = 128
DV = 128
WIN = 512
FRONT = 256
THETA = 10000.0
P = 128
NC_ = 8          # cores
SC = S // NC_    # 512 queries per core
KO = D // P      # 16 contraction chunks
NT = 10          # packed key tiles per core (2 front + 8 band)
NKC = NT * P     # 1280 packed key positions
NQTL = 4         # query tiles per core
NKT = 7          # key tiles per query tile

F32 = mybir.dt.float32
BF16 = mybir.dt.bfloat16
FP16 = mybir.dt.float16

try:
    import ml_dtypes

    ml_bf16 = ml_dtypes.bfloat16
except ImportError:  # pragma: no cover
    ml_bf16 = np.float32


def build_program():
    nc = bacc.Bacc(None, target_bir_lowering=False)

    x_d = nc.dram_tensor("xp", [P, KO, NKC], BF16, kind="ExternalInput")
    wq_d = nc.dram_tensor("wq", [P, 4, KO, 4 * DQK], BF16, kind="ExternalInput")
    wk_d = nc.dram_tensor("wk", [P, NKV, KO, DQK], BF16, kind="ExternalInput")
    wv_d = nc.dram_tensor("wv", [P, KO, NKV * DV], BF16, kind="ExternalInput")
    wo_d = nc.dram_tensor("wo", [P, 4, NH, SC], BF16, kind="ExternalInput")
    cos_d = nc.dram_tensor("cosd", [P, NKC], FP16, kind="ExternalInput")
    sin_d = nc.dram_tensor("sind", [P, NKC], FP16, kind="ExternalInput")
    b_d = nc.dram_tensor("bmask", [P, 11, P], BF16, kind="ExternalInput")
    rsel_d = nc.dram_tensor("rsel", [P, NT], BF16, kind="ExternalInput")
    y_d = nc.dram_tensor("y", [SC, D], BF16, kind="ExternalOutput")

    inv_sqrt_dqk = 1.0 / math.sqrt(DQK)
    qc0 = NKC - SC  # first packed col of this core's own queries (768)

    with tile.TileContext(nc) as tc:
        with (
            tc.tile_pool(name="persist", bufs=1) as persist,
            tc.tile_pool(name="ps", bufs=4, space="PSUM") as ps,
            tc.tile_pool(name="psO", bufs=2, space="PSUM") as psO,
            tc.tile_pool(name="psL", bufs=2, space="PSUM") as psL,
        ):
            # ---- persistent SBUF (lives through both phases) ----
            qT = persist.tile([P, NH, SC], BF16, tag="qT")
            kT = persist.tile([P, NKV, NKC], BF16, tag="kT")
            v_sb = persist.tile([P, NT, NKV * DV], BF16, tag="v")
            outT = persist.tile([P, NH, SC], BF16, tag="outT")
            b_sb = persist.tile([P, 11, P], BF16, tag="bm")
            rsel_sb = persist.tile([P, NT], BF16, tag="rsel")
            ones_row = persist.tile([1, P], BF16, tag="onesr")

            nc.vector.memset(ones_row[:], 1.0)

            def rope(dst, psrc, cosap, sinap, pool, w):
                """dst(bf16) = RoPE(psrc) in the paired [re(64); im(64)] basis.

                sw = [-im; re]; dst = psrc*cos + sw*sin.
                """
                sw = pool.tile([P, w], F32, tag="sw")
                nc.scalar.mul(sw[0:64], psrc[64:128], -1.0)
                nc.scalar.copy(sw[64:128], psrc[0:64])
                trc = pool.tile([P, w], F32, tag="trc")
                nc.vector.tensor_tensor(
                    trc[:], psrc, cosap, op=mybir.AluOpType.mult
                )
                nc.vector.tensor_tensor(
                    sw[:], sw[:], sinap, op=mybir.AluOpType.mult
                )
                nc.vector.tensor_tensor(
                    dst, trc[:], sw[:], op=mybir.AluOpType.add
                )

            # ---- Phase A: projections + RoPE (x/weights pool freed after) ----
            with tc.tile_pool(name="phA", bufs=1) as pa, tc.tile_pool(
                name="wqs", bufs=2
            ) as wqs, tc.tile_pool(name="ropep", bufs=3) as rp:
                x_sb = pa.tile([P, KO, NKC], BF16, tag="x")
                cos_sb = pa.tile([P, NKC], FP16, tag="cos")
                sin_sb = pa.tile([P, NKC], FP16, tag="sin")
                wk_sb = pa.tile([P, NKV, KO, DQK], BF16, tag="wk")
                wv_sb = pa.tile([P, KO, NKV * DV], BF16, tag="wv")

                nc.sync.dma_start(wk_sb[:, 0, 0:4], wk_d[:, 0, 0:4])
                nc.sync.dma_start(x_sb[:, 0:2, 0:512], x_d[:, 0:2, 0:512])
                nc.sync.dma_start(x_sb[:, 2:4, 0:512], x_d[:, 2:4, 0:512])
                nc.sync.dma_start(wk_sb[:, 0, 4:16], wk_d[:, 0, 4:16])
                for kg in range(4, KO, 4):
                    nc.sync.dma_start(
                        x_sb[:, kg : kg + 4, 0:512],
                        x_d[:, kg : kg + 4, 0:512],
                    )
                nc.sync.dma_start(cos_sb[:, 0:512], cos_d[:, 0:512])
                nc.sync.dma_start(sin_sb[:, 0:512], sin_d[:, 0:512])
                for _kvh in range(1, NKV):
                    nc.sync.dma_start(wk_sb[:, _kvh], wk_d[:, _kvh])
                nc.sync.dma_start(wv_sb[:], wv_d[:])
                for c0 in range(512, NKC, 512):
                    cw = min(512, NKC - c0)
                    nc.sync.dma_start(
                        x_sb[:, :, c0 : c0 + cw], x_d[:, :, c0 : c0 + cw]
                    )
                    nc.sync.dma_start(
                        cos_sb[:, c0 : c0 + cw], cos_d[:, c0 : c0 + cw]
                    )
                    nc.sync.dma_start(
                        sin_sb[:, c0 : c0 + cw], sin_d[:, c0 : c0 + cw]
                    )
                nc.sync.dma_start(b_sb[:], b_d[:])
                nc.sync.dma_start(rsel_sb[:], rsel_d[:])

                # k+v projection, chunk-outer so compute starts on chunk 0
                for c0 in range(0, NKC, 512):
                    cw = min(512, NKC - c0)
                    for kvh in range(NKV):
                        psk = ps.tile([P, SC], F32, tag="big", name="psk")
                        psk = psk[:, :cw]
                        for ko in range(KO):
                            nc.tensor.matmul(
                                psk,
                                wk_sb[:, kvh, ko, :],
                                x_sb[:, ko, c0 : c0 + cw],
                                start=(ko == 0),
                                stop=(ko == KO - 1),
                            )
                        rope(
                            kT[:, kvh, c0 : c0 + cw],
                            psk,
                            cos_sb[:, c0 : c0 + cw],
                            sin_sb[:, c0 : c0 + cw],
                            rp,
                            cw,
                        )
                    # v projection (natural [keys, dv], all 4 kv heads)
                    for t in range(c0 // P, min(NT, (c0 + cw) // P)):
                        psv = ps.tile([P, SC], F32, tag="big", name="psv")
                        psv = psv[:, : NKV * DV]
                        for ko in range(KO):
                            nc.tensor.matmul(
                                psv,
                                x_sb[:, ko, t * P : (t + 1) * P],
                                wv_sb[:, ko, :],
                                start=(ko == 0),
                                stop=(ko == KO - 1),
                            )
                        nc.vector.tensor_copy(v_sb[:, t, :], psv)

                # q projection + RoPE, streaming wq in 4-head groups
                for g in range(4):
                    wq_g = wqs.tile([P, KO, 4 * DQK], BF16, tag="wqg")
                    nc.sync.dma_start(wq_g[:], wq_d[:, g])
                    for hh in range(4):
                        h = 4 * g + hh
                        psq = ps.tile([P, SC], F32, tag="big", name="psq")
                        psq = psq[:, :SC]
                        for ko in range(KO):
                            nc.tensor.matmul(
                                psq,
                                wq_g[:, ko, hh * DQK : (hh + 1) * DQK],
                                x_sb[:, ko, qc0:NKC],
                                start=(ko == 0),
                                stop=(ko == KO - 1),
                            )
                        rope(
                            qT[:, h, :],
                            psq,
                            cos_sb[:, qc0:NKC],
                            sin_sb[:, qc0:NKC],
                            rp,
                            SC,
                        )

            # ---- Phase B: attention (transposed scores, per-head tile rows) --
            # Per key tile t, the q columns that can attend it:
            #   front tiles (t=0,1): all 512;  band tile b: qtl in [b-4, b].
            qr = {0: (0, SC), 1: (0, SC)}
            for b in range(8):
                lo = max(0, b - 4) * P
                hi = (min(3, b) + 1) * P
                qr[2 + b] = (lo, hi - lo)
            # phase C pools opened now so wo prefetch overlaps phase B
            pc = tc.alloc_tile_pool(name="phC", bufs=2)
            pcy = tc.alloc_tile_pool(name="phCy", bufs=4)
            with tc.tile_pool(name="phB", bufs=24) as pb, tc.tile_pool(
                name="phBs", bufs=3
            ) as pbs:
                tails = [None] * NH  # (psl, pso) pending normalize

                def tail_recip(h):
                    psl, pso = tails[h]
                    irl = pbs.tile([1, SC], F32, tag="irl", name="irl")
                    nc.vector.reciprocal_approx_fast(irl[:], psl[:])
                    return irl

                def tail_mid(h, irl):
                    lrow = pbs.tile([1, SC], BF16, tag="lrow", name="lrow")
                    nc.scalar.copy(lrow[:], irl[:])
                    return lrow

                def tail_apply(h, lrow):
                    _, pso = tails[h]
                    psbc = psL.tile([P, SC], F32, tag="l", name="psbc")
                    nc.tensor.matmul(
                        psbc[:], ones_row[:], lrow[:], start=True, stop=True
                    )
                    rlbc = pbs.tile([P, SC], BF16, tag="rlbc", name="rlbc")
                    nc.scalar.copy(rlbc[:], psbc[:])
                    nc.vector.tensor_tensor(
                        outT[:, h, :], pso[:], rlbc[:], op=mybir.AluOpType.mult
                    )
                    tails[h] = None

                # masked 128-col blocks per tile: (block index, col offset)
                mask_blocks = {0: [(0, 0)], 1: [(1, 0), (2, P)]}
                for b in range(8):
                    off = (b if b < 4 else 0) * P
                    mask_blocks[2 + b] = [(3 + b, off)]

                def emit_scores(h):
                    kvh = h // NREP
                    pTs = []
                    for ti in range(NT):
                        q0, qw = qr[ti]
                        pst = ps.tile([P, SC], F32, tag="big", name="pst")
                        pst = pst[:, :qw]
                        nc.tensor.matmul(
                            pst,
                            kT[:, kvh, ti * P : (ti + 1) * P],
                            qT[:, h, q0 : q0 + qw],
                            start=True,
                            stop=True,
                        )
                        pTt = pb.tile([P, SC], BF16, tag="pT", name="pTt")
                        pTt = pTt[:, :qw]
                        nc.scalar.activation(
                            pTt,
                            pst,
                            mybir.ActivationFunctionType.Exp,
                            scale=inv_sqrt_dqk,
                        )
                        for blk, off in mask_blocks[ti]:
                            nc.vector.tensor_tensor(
                                pTt[:, off : off + P],
                                pTt[:, off : off + P],
                                b_sb[:, blk, :],
                                op=mybir.AluOpType.mult,
                            )
                        pTs.append(pTt)
                    return pTs

                def emit_la(h, pTs):
                    kvh = h // NREP
                    pso = psO.tile([P, SC], F32, tag="o", name="pso")
                    psl = psL.tile([1, SC], F32, tag="l", name="psl")
                    for ti in range(NT):
                        q0, qw = qr[ti]
                        nc.tensor.matmul(
                            psl[:, q0 : q0 + qw],
                            rsel_sb[:, ti : ti + 1],
                            pTs[ti],
                            start=(ti == 0),
                            stop=(ti == NT - 1),
                            skip_group_check=True,
                        )
                    for ti in range(NT):
                        q0, qw = qr[ti]
                        nc.tensor.matmul(
                            pso[:, q0 : q0 + qw],
                            v_sb[:, ti, kvh * DV : (kvh + 1) * DV],
                            pTs[ti],
                            start=(ti == 0),
                            stop=(ti == NT - 1),
                            skip_group_check=True,
                        )
                    tails[h] = (psl, pso)

                for hp in range(0, NH, 2):
                    lr0 = lr1 = None
                    if hp > 0:
                        lr0 = tail_mid(hp - 2, tail_recip(hp - 2))
                        lr1 = tail_mid(hp - 1, tail_recip(hp - 1))
                    pTs0 = emit_scores(hp)
                    pTs1 = emit_scores(hp + 1)
                    if hp > 0:
                        tail_apply(hp - 2, lr0)
                        tail_apply(hp - 1, lr1)
                    emit_la(hp, pTs0)
                    emit_la(hp + 1, pTs1)
                tail_apply(NH - 2, tail_mid(NH - 2, tail_recip(NH - 2)))
                tail_apply(NH - 1, tail_mid(NH - 1, tail_recip(NH - 1)))

            # ---- Phase C: y = outT^T @ wo (stream wo in n-chunks) ----
            y_tiles = [
                pcy.tile([P, D], BF16, tag="y", name=f"y{i}")
                for i in range(NQTL)
            ]
            for ncl in range(4):
                wo_g = pc.tile([P, NH, SC], BF16, tag="wog", name="wo_g")
                nc.sync.dma_start(wo_g[:], wo_d[:, ncl])
                for qtl in range(NQTL):
                    psy = ps.tile([P, SC], F32, tag="big", name="psy")
                    for h in range(NH):
                        nc.tensor.matmul(
                            psy[:],
                            outT[:, h, qtl * P : (qtl + 1) * P],
                            wo_g[:, h, :],
                            start=(h == 0),
                            stop=(h == NH - 1),
                        )
                    nc.vector.tensor_copy(
                        y_tiles[qtl][:, ncl * SC : (ncl + 1) * SC], psy[:]
                    )
                    nc.sync.dma_start(
                        y_d[
                            qtl * P : (qtl + 1) * P,
                            ncl * SC : (ncl + 1) * SC,
                        ],
                        y_tiles[qtl][:, ncl * SC : (ncl + 1) * SC],
                    )
            pcy.release()
            pc.release()

    return nc


_PROGRAM = None


def _get_program():
    global _PROGRAM
    if _PROGRAM is None:
        _PROGRAM = build_program()
        _PROGRAM.finalize()
    return _PROGRAM


def _host_inputs(x, wq, wk, wv, wo):
    """Per-core input packing (all arrays contiguous, uniform shapes)."""
    x2 = np.asarray(x, np.float32).reshape(S, D)
    xT = np.ascontiguousarray(x2.T)  # [D, S]
    xr = xT.reshape(KO, P, S)  # [ko, p, s]

    # paired RoPE basis permutation within each head
    perm = np.concatenate([np.arange(0, DQK, 2), np.arange(1, DQK, 2)])
    wq_p = np.asarray(wq, np.float32).reshape(D, NH, DQK)[:, :, perm]
    wk_p = np.asarray(wk, np.float32).reshape(D, NKV, DQK)[:, :, perm]
    wv_r = np.asarray(wv, np.float32).reshape(D, NKV * DV)
    wo_r = np.asarray(wo, np.float32).reshape(NH, DV, D)

    # device layouts independent of core
    wq_dev = np.ascontiguousarray(
        wq_p.reshape(KO, P, NH, DQK)  # [ko, p, h, dqk]
        .reshape(KO, P, 4, 4 * DQK)  # group 4 heads
        .transpose(1, 2, 0, 3)  # [p, g, ko, 4*dqk]
    ).astype(ml_bf16)
    wk_dev = np.ascontiguousarray(
        wk_p.reshape(KO, P, NKV, DQK).transpose(1, 2, 0, 3)
    ).astype(ml_bf16)
    wv_dev = np.ascontiguousarray(
        wv_r.reshape(KO, P, NKV * DV).transpose(1, 0, 2)
    ).astype(ml_bf16)
    wo_dev = np.ascontiguousarray(
        wo_r.reshape(NH, DV, 4, SC).transpose(1, 2, 0, 3)  # [dv, ncl, h, sc]
    ).astype(ml_bf16)

    inv_freq = 1.0 / (THETA ** (np.arange(0, DQK, 2)[: DQK // 2] / DQK))

    in_maps = []
    for c in range(NC_):
        qlo = c * SC
        band_lo = qlo - WIN
        # packed key positions; garbage (pos<0) -> position 0, zero x
        pos = np.empty(NKC, np.int64)
        pos[: FRONT] = np.arange(FRONT)
        pos[FRONT:] = band_lo + np.arange(NKC - FRONT)
        valid = pos >= 0
        pos_c = np.where(valid, pos, 0)

        xp = xr[:, :, pos_c] * valid[None, None, :]  # [ko, p, nkc]
        if c == 0:
            # front tiles are dead on core 0 (band covers them); zero x so
            # their v projection is 0 and unmasked pT blocks are harmless
            xp[:, :, :FRONT] = 0.0
        xp = np.ascontiguousarray(xp.transpose(1, 0, 2)).astype(ml_bf16)

        ang = np.outer(pos_c.astype(np.float64), inv_freq)  # (nkc, 64)
        cos_h = np.cos(ang).T.astype(np.float32)  # (64, nkc)
        sin_h = np.sin(ang).T.astype(np.float32)
        cos_p = np.ascontiguousarray(
            np.concatenate([cos_h, cos_h], 0)
        ).astype(np.float16)
        sin_p = np.ascontiguousarray(
            np.concatenate([sin_h, sin_h], 0)
        ).astype(np.float16)

        # Sparse 128x128 mask blocks (only blocks non-full on some core):
        #   blk 0: (front0, qtl0)  blk 1: (front1, qtl0)  blk 2: (front1, qtl1)
        #   blk 3+b: (band b, qtl b) for b<4 else (band b, qtl b-4)
        # Front tiles use only front_ok, band tiles only the band condition.
        r = np.arange(P)[:, None]
        B = np.zeros((P, 11, P), np.float32)
        blocks = [(0, 0), (1, 0), (1, 1)] + [
            (2 + b, b if b < 4 else b - 4) for b in range(8)
        ]
        for blk, (t, qtl) in enumerate(blocks):
            qpos = qlo + qtl * P + np.arange(P)[None, :]
            if t < 2:
                kpos = t * P + r
                allowed = (kpos < FRONT) & (kpos <= qpos - WIN)
            else:
                kpos = band_lo + (t - 2) * P + r
                allowed = (kpos >= 0) & (kpos <= qpos) & (kpos > qpos - WIN)
            B[:, blk, :] = allowed
        Bp = np.ascontiguousarray(B).astype(ml_bf16)
        # row-select for the lsum reduction: excludes padding rows and,
        # on cores 0-1 ... front rows are excluded only when front tiles
        # are dead (core 0, whose front x is zeroed; band covers front).
        rsel = np.zeros((P, NT), np.float32)
        for t in range(NT):
            if t < 2:
                rsel[:, t] = 0.0 if c == 0 else 1.0
            else:
                kpos = band_lo + (t - 2) * P + r[:, 0]
                rsel[:, t] = (kpos >= 0).astype(np.float32)
        rsel_p = np.ascontiguousarray(rsel).astype(ml_bf16)

        in_maps.append(
            {
                "xp": xp,
                "wq": wq_dev,
                "wk": wk_dev,
                "wv": wv_dev,
                "wo": wo_dev,
                "cosd": cos_p,
                "sind": sin_p,
                "bmask": Bp,
                "rsel": rsel_p,
            }
        )
    return in_maps


def kernel(x, wq, wk, wv, wo, _trace=False, _trace_kwargs=None):
    nc = _get_program()
    in_maps = _host_inputs(x, wq, wk, wv, wo)
    res = run_bass_kernel_spmd(
        nc, in_maps, list(range(NC_)), trace=_trace, **(_trace_kwargs or {})
    )
    y = np.concatenate(
        [np.asarray(r["y"], np.float32) for r in res.results], axis=0
    )
    out = y.reshape(1, S, D)
    if _trace:
        return out, res
    return out


# revision 30
# speedup vs baseline: 1.0171x; 1.0171x over previous
"""Sparse-attention (sliding window 512 + front 256) Trainium2 kernel.

Sequence-sharded across 8 NeuronCores: core c owns queries [512c, 512c+512)
and computes ALL 16 heads for them, including the full output projection, so
per-core output is a disjoint y slice [512, 2048] (no cross-core reduction).

Layout: scores are computed TRANSPOSED (sT[k, q] = kT_tile^T @ qT), so the
exp() output is already the [keys, q] rhs operand the attn@V matmul needs --
no PE transposes and no PSUM->SBUF staging copies. The softmax key-dim sum is
a row-select-vector matmul on the PE; 1/lsum (reciprocal_approx_fast) is
broadcast across partitions with a rank-1 ones matmul and applied to the
attn@V PSUM output by the DVE on its way to SBUF.

Uniform single program across cores; per-core variation is data only:
  - packed key layout (NT=10 tiles): cols 0:256 front, cols 256:1280 band
    positions [qlo-512, qlo+512), zero-x for padding (and for core 0's dead
    front tiles, which its band range covers);
  - 11 sparse 128x128 {0,1} mask blocks (the only blocks not full on every
    core: anti/causal window edges + core-0/1 front edge cases), applied
    multiplicatively to exp(scores) -- front tiles use only the front_ok
    condition, band tiles only the band condition (disjoint), so duplicated
    front/band keys on cores 0-1 never double count;
  - a [128, NT] row-select vector excluding padding/dead rows from lsum.

Phase pipeline: A) k/v/q projections + paired-basis RoPE, chunk-ordered so
compute starts with the first x DMA; B) attention in head pairs (20 score
tiles batched ahead of the lsum/attn@V runs, normalize tail software-
pipelined one pair behind); C) y projection streaming wo in n-chunks,
h-accumulation letting the scheduler overlap C with B's last heads.
"""
import math
import sys

import numpy as np

sys.path.insert(0, "/opt/trn_rl_repo")

import concourse.bass as bass
from concourse import bacc
import concourse.mybir as mybir
import concourse.tile as tile
from concourse.bass_utils import run_bass_kernel_spmd

# Problem constants (hardcoded per contract)
S = 4096
D = 2048
NH = 16
NKV = 4
NREP = NH // NKV
DQK = 128
DV = 128
WIN = 512
FRONT = 256
THETA = 10000.0
P = 128
NC_ = 8          # cores
SC = S // NC_    # 512 queries per core
KO = D // P      # 16 contraction chunks
NT = 10          # packed key tiles per core (2 front + 8 band)
NKC = NT * P     # 1280 packed key positions
NQTL = 4         # query tiles per core
NKT = 7          # key tiles per query tile

F32 = mybir.dt.float32
BF16 = mybir.dt.bfloat16
FP16 = mybir.dt.float16

try:
    import ml_dtypes

    ml_bf16 = ml_dtypes.bfloat16
except ImportError:  # pragma: no cover
    ml_bf16 = np.float32


def build_program():
    nc = bacc.Bacc(None, target_bir_lowering=False)

    x_d = nc.dram_tensor("xp", [P, KO, NKC], BF16, kind="ExternalInput")
    wq_d = nc.dram_tensor("wq", [P, 4, KO, 4 * DQK], BF16, kind="ExternalInput")
    wk_d = nc.dram_tensor("wk", [P, NKV, KO, DQK], BF16, kind="ExternalInput")
    wv_d = nc.dram_tensor("wv", [P, KO, NKV * DV], BF16, kind="ExternalInput")
    wo_d = nc.dram_tensor("wo", [P, 4, NH, SC], BF16, kind="ExternalInput")
    cos_d = nc.dram_tensor("cosd", [P, NKC], FP16, kind="ExternalInput")
    sin_d = nc.dram_tensor("sind", [P, NKC], FP16, kind="ExternalInput")
    b_d = nc.dram_tensor("bmask", [P, 11, P], BF16, kind="ExternalInput")
    rsel_d = nc.dram_tensor("rsel", [P, NT], BF16, kind="ExternalInput")
    y_d = nc.dram_tensor("y", [SC, D], BF16, kind="ExternalOutput")

    inv_sqrt_dqk = 1.0 / math.sqrt(DQK)
    qc0 = NKC - SC  # first packed col of this core's own queries (768)

    with tile.TileContext(nc) as tc:
        with (
            tc.tile_pool(name="persist", bufs=1) as persist,
            tc.tile_pool(name="ps", bufs=4, space="PSUM") as ps,
            tc.tile_pool(name="psO", bufs=2, space="PSUM") as psO,
            tc.tile_pool(name="psL", bufs=2, space="PSUM") as psL,
        ):
            # ---- persistent SBUF (lives through both phases) ----
            qT = persist.tile([P, NH, SC], BF16, tag="qT")
            kT = persist.tile([P, NKV, NKC], BF16, tag="kT")
            v_sb = persist.tile([P, NT, NKV * DV], BF16, tag="v")
            outT = persist.tile([P, NH, SC], BF16, tag="outT")
            b_sb = persist.tile([P, 11, P], BF16, tag="bm")
            rsel_sb = persist.tile([P, NT], BF16, tag="rsel")
            ones_row = persist.tile([1, P], BF16, tag="onesr")

            nc.vector.memset(ones_row[:], 1.0)

            def rope(dst, psrc, cosap, sinap, pool, w):
                """dst(bf16) = RoPE(psrc) in the paired [re(64); im(64)] basis.

                sw = [-im; re]; dst = psrc*cos + sw*sin.
                """
                sw = pool.tile([P, w], F32, tag="sw")
                nc.scalar.mul(sw[0:64], psrc[64:128], -1.0)
                nc.scalar.copy(sw[64:128], psrc[0:64])
                trc = pool.tile([P, w], F32, tag="trc")
                nc.vector.tensor_tensor(
                    trc[:], psrc, cosap, op=mybir.AluOpType.mult
                )
                nc.vector.tensor_tensor(
                    sw[:], sw[:], sinap, op=mybir.AluOpType.mult
                )
                nc.vector.tensor_tensor(
                    dst, trc[:], sw[:], op=mybir.AluOpType.add
                )

            # ---- Phase A: projections + RoPE (x/weights pool freed after) ----
            with tc.tile_pool(name="phA", bufs=1) as pa, tc.tile_pool(
                name="wqs", bufs=2
            ) as wqs, tc.tile_pool(name="ropep", bufs=3) as rp:
                x_sb = pa.tile([P, KO, NKC], BF16, tag="x")
                cos_sb = pa.tile([P, NKC], FP16, tag="cos")
                sin_sb = pa.tile([P, NKC], FP16, tag="sin")
                wk_sb = pa.tile([P, NKV, KO, DQK], BF16, tag="wk")
                wv_sb = pa.tile([P, KO, NKV * DV], BF16, tag="wv")

                nc.sync.dma_start(wk_sb[:, 0, 0:4], wk_d[:, 0, 0:4])
                nc.sync.dma_start(x_sb[:, 0:2, 0:512], x_d[:, 0:2, 0:512])
                nc.sync.dma_start(x_sb[:, 2:4, 0:512], x_d[:, 2:4, 0:512])
                nc.sync.dma_start(wk_sb[:, 0, 4:16], wk_d[:, 0, 4:16])
                for kg in range(4, KO, 4):
                    nc.sync.dma_start(
                        x_sb[:, kg : kg + 4, 0:512],
                        x_d[:, kg : kg + 4, 0:512],
                    )
                nc.sync.dma_start(cos_sb[:, 0:512], cos_d[:, 0:512])
                nc.sync.dma_start(sin_sb[:, 0:512], sin_d[:, 0:512])
                for _kvh in range(1, NKV):
                    nc.sync.dma_start(wk_sb[:, _kvh], wk_d[:, _kvh])
                nc.sync.dma_start(wv_sb[:], wv_d[:])
                for c0 in range(512, NKC, 512):
                    cw = min(512, NKC - c0)
                    nc.sync.dma_start(
                        x_sb[:, :, c0 : c0 + cw], x_d[:, :, c0 : c0 + cw]
                    )
                    nc.sync.dma_start(
                        cos_sb[:, c0 : c0 + cw], cos_d[:, c0 : c0 + cw]
                    )
                    nc.sync.dma_start(
                        sin_sb[:, c0 : c0 + cw], sin_d[:, c0 : c0 + cw]
                    )
                nc.sync.dma_start(b_sb[:], b_d[:])
                nc.sync.dma_start(rsel_sb[:], rsel_d[:])

                # k+v projection, chunk-outer so compute starts on chunk 0
                for c0 in range(0, NKC, 512):
                    cw = min(512, NKC - c0)
                    for kvh in range(NKV):
                        psk = ps.tile([P, SC], F32, tag="big", name="psk")
                        psk = psk[:, :cw]
                        for ko in range(KO):
                            nc.tensor.matmul(
                                psk,
                                wk_sb[:, kvh, ko, :],
                                x_sb[:, ko, c0 : c0 + cw],
                                start=(ko == 0),
                                stop=(ko == KO - 1),
                            )
                        rope(
                            kT[:, kvh, c0 : c0 + cw],
                            psk,
                            cos_sb[:, c0 : c0 + cw],
                            sin_sb[:, c0 : c0 + cw],
                            rp,
                            cw,
                        )
                    # v projection (natural [keys, dv], all 4 kv heads)
                    for t in range(c0 // P, min(NT, (c0 + cw) // P)):
                        psv = ps.tile([P, SC], F32, tag="big", name="psv")
                        psv = psv[:, : NKV * DV]
                        for ko in range(KO):
                            nc.tensor.matmul(
                                psv,
                                x_sb[:, ko, t * P : (t + 1) * P],
                                wv_sb[:, ko, :],
                                start=(ko == 0),
                                stop=(ko == KO - 1),
                            )
                        nc.vector.tensor_copy(v_sb[:, t, :], psv)

                # q projection + RoPE, streaming wq in 4-head groups
                for g in range(4):
                    wq_g = wqs.tile([P, KO, 4 * DQK], BF16, tag="wqg")
                    nc.sync.dma_start(wq_g[:], wq_d[:, g])
                    for hh in range(4):
                        h = 4 * g + hh
                        psq = ps.tile([P, SC], F32, tag="big", name="psq")
                        psq = psq[:, :SC]
                        for ko in range(KO):
                            nc.tensor.matmul(
                                psq,
                                wq_g[:, ko, hh * DQK : (hh + 1) * DQK],
                                x_sb[:, ko, qc0:NKC],
                                start=(ko == 0),
                                stop=(ko == KO - 1),
                            )
                        rope(
                            qT[:, h, :],
                            psq,
                            cos_sb[:, qc0:NKC],
                            sin_sb[:, qc0:NKC],
                            rp,
                            SC,
                        )

            # ---- Phase B: attention (transposed scores, per-head tile rows) --
            # Per key tile t, the q columns that can attend it:
            #   front tiles (t=0,1): all 512;  band tile b: qtl in [b-4, b].
            qr = {0: (0, SC), 1: (0, SC)}
            for b in range(8):
                lo = max(0, b - 4) * P
                hi = (min(3, b) + 1) * P
                qr[2 + b] = (lo, hi - lo)
            # phase C pools opened now so wo prefetch overlaps phase B
            pc = tc.alloc_tile_pool(name="phC", bufs=2)
            pcy = tc.alloc_tile_pool(name="phCy", bufs=4)
            with tc.tile_pool(name="phB", bufs=22) as pb, tc.tile_pool(
                name="phBs", bufs=3
            ) as pbs:
                tails = [None] * NH  # (psl, pso) pending normalize

                def tail_recip(h):
                    psl, pso = tails[h]
                    irl = pbs.tile([1, SC], F32, tag="irl", name="irl")
                    nc.vector.reciprocal_approx_fast(irl[:], psl[:])
                    return irl

                def tail_mid(h, irl):
                    lrow = pbs.tile([1, SC], BF16, tag="lrow", name="lrow")
                    nc.scalar.copy(lrow[:], irl[:])
                    return lrow

                def tail_apply(h, lrow):
                    _, pso = tails[h]
                    psbc = ps.tile([P, SC], F32, tag="big", name="psbc")
                    nc.tensor.matmul(
                        psbc[:], ones_row[:], lrow[:], start=True, stop=True
                    )
                    rlbc = pbs.tile([P, SC], BF16, tag="rlbc", name="rlbc")
                    nc.scalar.copy(rlbc[:], psbc[:])
                    nc.vector.tensor_tensor(
                        outT[:, h, :], pso[:], rlbc[:], op=mybir.AluOpType.mult
                    )
                    tails[h] = None

                # masked 128-col blocks per tile: (block index, col offset)
                mask_blocks = {0: [(0, 0)], 1: [(1, 0), (2, P)]}
                for b in range(8):
                    off = (b if b < 4 else 0) * P
                    mask_blocks[2 + b] = [(3 + b, off)]

                def emit_scores(h):
                    kvh = h // NREP
                    pTs = []
                    for ti in range(NT):
                        q0, qw = qr[ti]
                        pst = ps.tile([P, SC], F32, tag="big", name="pst")
                        pst = pst[:, :qw]
                        nc.tensor.matmul(
                            pst,
                            kT[:, kvh, ti * P : (ti + 1) * P],
                            qT[:, h, q0 : q0 + qw],
                            start=True,
                            stop=True,
                        )
                        pTt = pb.tile([P, SC], BF16, tag="pT", name="pTt")
                        pTt = pTt[:, :qw]
                        nc.scalar.activation(
                            pTt,
                            pst,
                            mybir.ActivationFunctionType.Exp,
                            scale=inv_sqrt_dqk,
                        )
                        for blk, off in mask_blocks[ti]:
                            nc.vector.tensor_tensor(
                                pTt[:, off : off + P],
                                pTt[:, off : off + P],
                                b_sb[:, blk, :],
                                op=mybir.AluOpType.mult,
                            )
                        pTs.append(pTt)
                    return pTs

                def emit_la(h, pTs):
                    kvh = h // NREP
                    pso = psO.tile([P, SC], F32, tag="o", name="pso")
                    psl = psL.tile([1, SC], F32, tag="l", name="psl")
                    for ti in range(NT):
                        q0, qw = qr[ti]
                        nc.tensor.matmul(
                            psl[:, q0 : q0 + qw],
                            rsel_sb[:, ti : ti + 1],
                            pTs[ti],
                            start=(ti == 0),
                            stop=(ti == NT - 1),
                            skip_group_check=True,
                        )
                    for ti in range(NT):
                        q0, qw = qr[ti]
                        nc.tensor.matmul(
                            pso[:, q0 : q0 + qw],
                            v_sb[:, ti, kvh * DV : (kvh + 1) * DV],
                            pTs[ti],
                            start=(ti == 0),
                            stop=(ti == NT - 1),
                            skip_group_check=True,
                        )
                    tails[h] = (psl, pso)

                for hp in range(0, NH, 2):
                    lr0 = lr1 = None
                    if hp > 0:
                        lr0 = tail_mid(hp - 2, tail_recip(hp - 2))
                        lr1 = tail_mid(hp - 1, tail_recip(hp - 1))
                    pTs0 = emit_scores(hp)
                    pTs1 = emit_scores(hp + 1)
                    if hp > 0:
                        tail_apply(hp - 2, lr0)
                        tail_apply(hp - 1, lr1)
                    emit_la(hp, pTs0)
                    emit_la(hp + 1, pTs1)
                tail_apply(NH - 2, tail_mid(NH - 2, tail_recip(NH - 2)))
                tail_apply(NH - 1, tail_mid(NH - 1, tail_recip(NH - 1)))

            # ---- Phase C: y = outT^T @ wo (stream wo in n-chunks) ----
            y_tiles = [
                pcy.tile([P, D], BF16, tag="y", name=f"y{i}")
                for i in range(NQTL)
            ]
            for ncl in range(4):
                wo_g = pc.tile([P, NH, SC], BF16, tag="wog", name="wo_g")
                nc.sync.dma_start(wo_g[:], wo_d[:, ncl])
                for qtl in range(NQTL):
                    psy = ps.tile([P, SC], F32, tag="big", name="psy")
                    for h in range(NH):
                        nc.tensor.matmul(
                            psy[:],
                            outT[:, h, qtl * P : (qtl + 1) * P],
                            wo_g[:, h, :],
                            start=(h == 0),
                            stop=(h == NH - 1),
                        )
                    nc.vector.tensor_copy(
                        y_tiles[qtl][:, ncl * SC : (ncl + 1) * SC], psy[:]
                    )
                    nc.sync.dma_start(
                        y_d[
                            qtl * P : (qtl + 1) * P,
                            ncl * SC : (ncl + 1) * SC,
                        ],
                        y_tiles[qtl][:, ncl * SC : (ncl + 1) * SC],
                    )
            pcy.release()
            pc.release()

    return nc


_PROGRAM = None


def _get_program():
    global _PROGRAM
    if _PROGRAM is None:
        _PROGRAM = build_program()
        _PROGRAM.finalize()
    return _PROGRAM


def _host_inputs(x, wq, wk, wv, wo):
    """Per-core input packing (all arrays contiguous, uniform shapes)."""
    x2 = np.asarray(x, np.float32).reshape(S, D)
    xT = np.ascontiguousarray(x2.T)  # [D, S]
    xr = xT.reshape(KO, P, S)  # [ko, p, s]

    # paired RoPE basis permutation within each head
    perm = np.concatenate([np.arange(0, DQK, 2), np.arange(1, DQK, 2)])
    wq_p = np.asarray(wq, np.float32).reshape(D, NH, DQK)[:, :, perm]
    wk_p = np.asarray(wk, np.float32).reshape(D, NKV, DQK)[:, :, perm]
    wv_r = np.asarray(wv, np.float32).reshape(D, NKV * DV)
    wo_r = np.asarray(wo, np.float32).reshape(NH, DV, D)

    # device layouts independent of core
    wq_dev = np.ascontiguousarray(
        wq_p.reshape(KO, P, NH, DQK)  # [ko, p, h, dqk]
        .reshape(KO, P, 4, 4 * DQK)  # group 4 heads
        .transpose(1, 2, 0, 3)  # [p, g, ko, 4*dqk]
    ).astype(ml_bf16)
    wk_dev = np.ascontiguousarray(
        wk_p.reshape(KO, P, NKV, DQK).transpose(1, 2, 0, 3)
    ).astype(ml_bf16)
    wv_dev = np.ascontiguousarray(
        wv_r.reshape(KO, P, NKV * DV).transpose(1, 0, 2)
    ).astype(ml_bf16)
    wo_dev = np.ascontiguousarray(
        wo_r.reshape(NH, DV, 4, SC).transpose(1, 2, 0, 3)  # [dv, ncl, h, sc]
    ).astype(ml_bf16)

    inv_freq = 1.0 / (THETA ** (np.arange(0, DQK, 2)[: DQK // 2] / DQK))

    in_maps = []
    for c in range(NC_):
        qlo = c * SC
        band_lo = qlo - WIN
        # packed key positions; garbage (pos<0) -> position 0, zero x
        pos = np.empty(NKC, np.int64)
        pos[: FRONT] = np.arange(FRONT)
        pos[FRONT:] = band_lo + np.arange(NKC - FRONT)
        valid = pos >= 0
        pos_c = np.where(valid, pos, 0)

        xp = xr[:, :, pos_c] * valid[None, None, :]  # [ko, p, nkc]
        if c == 0:
            # front tiles are dead on core 0 (band covers them); zero x so
            # their v projection is 0 and unmasked pT blocks are harmless
            xp[:, :, :FRONT] = 0.0
        xp = np.ascontiguousarray(xp.transpose(1, 0, 2)).astype(ml_bf16)

        ang = np.outer(pos_c.astype(np.float64), inv_freq)  # (nkc, 64)
        cos_h = np.cos(ang).T.astype(np.float32)  # (64, nkc)
        sin_h = np.sin(ang).T.astype(np.float32)
        cos_p = np.ascontiguousarray(
            np.concatenate([cos_h, cos_h], 0)
        ).astype(np.float16)
        sin_p = np.ascontiguousarray(
            np.concatenate([sin_h, sin_h], 0)
        ).astype(np.float16)

        # Sparse 128x128 mask blocks (only blocks non-full on some core):
        #   blk 0: (front0, qtl0)  blk 1: (front1, qtl0)  blk 2: (front1, qtl1)
        #   blk 3+b: (band b, qtl b) for b<4 else (band b, qtl b-4)
        # Front tiles use only front_ok, band tiles only the band condition.
        r = np.arange(P)[:, None]
        B = np.zeros((P, 11, P), np.float32)
        blocks = [(0, 0), (1, 0), (1, 1)] + [
            (2 + b, b if b < 4 else b - 4) for b in range(8)
        ]
        for blk, (t, qtl) in enumerate(blocks):
            qpos = qlo + qtl * P + np.arange(P)[None, :]
            if t < 2:
                kpos = t * P + r
                allowed = (kpos < FRONT) & (kpos <= qpos - WIN)
            else:
                kpos = band_lo + (t - 2) * P + r
                allowed = (kpos >= 0) & (kpos <= qpos) & (kpos > qpos - WIN)
            B[:, blk, :] = allowed
        Bp = np.ascontiguousarray(B).astype(ml_bf16)
        # row-select for the lsum reduction: excludes padding rows and,
        # on cores 0-1 ... front rows are excluded only when front tiles
        # are dead (core 0, whose front x is zeroed; band covers front).
        rsel = np.zeros((P, NT), np.float32)
        for t in range(NT):
            if t < 2:
                rsel[:, t] = 0.0 if c == 0 else 1.0
            else:
                kpos = band_lo + (t - 2) * P + r[:, 0]
                rsel[:, t] = (kpos >= 0).astype(np.float32)
        rsel_p = np.ascontiguousarray(rsel).astype(ml_bf16)

        in_maps.append(
            {
                "xp": xp,
                "wq": wq_dev,
                "wk": wk_dev,
                "wv": wv_dev,
                "wo": wo_dev,
                "cosd": cos_p,
                "sind": sin_p,
                "bmask": Bp,
                "rsel": rsel_p,
            }
        )
    return in_maps


def kernel(x, wq, wk, wv, wo, _trace=False, _trace_kwargs=None):
    nc = _get_program()
    in_maps = _host_inputs(x, wq, wk, wv, wo)
    res = run_bass_kernel_spmd(
        nc, in_maps, list(range(NC_)), trace=_trace, **(_trace_kwargs or {})
    )
    y = np.concatenate(
        [np.asarray(r["y"], np.float32) for r in res.results], axis=0
    )
    out = y.reshape(1, S, D)
    if _trace:
        return out, res
    return out


# revision 31
# speedup vs baseline: 1.0727x; 1.0546x over previous
"""Sparse-attention (sliding window 512 + front 256) Trainium2 kernel.

Sequence-sharded across 8 NeuronCores: core c owns queries [512c, 512c+512)
and computes ALL 16 heads for them, including the full output projection, so
per-core output is a disjoint y slice [512, 2048] (no cross-core reduction).

Layout: scores are computed TRANSPOSED (sT[k, q] = kT_tile^T @ qT), so the
exp() output is already the [keys, q] rhs operand the attn@V matmul needs --
no PE transposes and no PSUM->SBUF staging copies. The softmax key-dim sum is
a row-select-vector matmul on the PE; 1/lsum (reciprocal_approx_fast) is
broadcast across partitions with a rank-1 ones matmul and applied to the
attn@V PSUM output by the DVE on its way to SBUF.

Uniform single program across cores; per-core variation is data only:
  - packed key layout (NT=10 tiles): cols 0:256 front, cols 256:1280 band
    positions [qlo-512, qlo+512), zero-x for padding (and for core 0's dead
    front tiles, which its band range covers);
  - 11 sparse 128x128 {0,1} mask blocks (the only blocks not full on every
    core: anti/causal window edges + core-0/1 front edge cases), applied
    multiplicatively to exp(scores) -- front tiles use only the front_ok
    condition, band tiles only the band condition (disjoint), so duplicated
    front/band keys on cores 0-1 never double count;
  - a [128, NT] row-select vector excluding padding/dead rows from lsum.

Phase pipeline: A) k/v/q projections + paired-basis RoPE, chunk-ordered so
compute starts with the first x DMA; B) attention in head pairs (20 score
tiles batched ahead of the lsum/attn@V runs, normalize tail software-
pipelined one pair behind); C) y projection streaming wo in n-chunks,
h-accumulation letting the scheduler overlap C with B's last heads.
"""
import math
import sys

import numpy as np

sys.path.insert(0, "/opt/trn_rl_repo")

import concourse.bass as bass
from concourse import bacc
import concourse.mybir as mybir
import concourse.tile as tile
from concourse.bass_utils import run_bass_kernel_spmd

# Problem constants (hardcoded per contract)
S = 4096
D = 2048
NH = 16
NKV = 4
NREP = NH // NKV
DQK = 128
DV = 128
WIN = 512
FRONT = 256
THETA = 10000.0
P = 128
NC_ = 8          # cores
SC = S // NC_    # 512 queries per core
KO = D // P      # 16 contraction chunks
NT = 10          # packed key tiles per core (2 front + 8 band)
NKC = NT * P     # 1280 packed key positions
NQTL = 4         # query tiles per core
NKT = 7          # key tiles per query tile

F32 = mybir.dt.float32
BF16 = mybir.dt.bfloat16
FP16 = mybir.dt.float16

try:
    import ml_dtypes

    ml_bf16 = ml_dtypes.bfloat16
except ImportError:  # pragma: no cover
    ml_bf16 = np.float32


def build_program():
    nc = bacc.Bacc(None, target_bir_lowering=False)

    x_d = nc.dram_tensor("xp", [P, KO, NKC], BF16, kind="ExternalInput")
    wq_d = nc.dram_tensor("wq", [P, 4, KO, 4 * DQK], BF16, kind="ExternalInput")
    wk_d = nc.dram_tensor("wk", [P, NKV, KO, DQK], BF16, kind="ExternalInput")
    wv_d = nc.dram_tensor("wv", [P, KO, NKV * DV], BF16, kind="ExternalInput")
    wo_d = nc.dram_tensor("wo", [P, 4, NH, SC], BF16, kind="ExternalInput")
    cos_d = nc.dram_tensor("cosd", [P, NKC], FP16, kind="ExternalInput")
    sin_d = nc.dram_tensor("sind", [P, NKC], FP16, kind="ExternalInput")
    b_d = nc.dram_tensor("bmask", [P, 11, P], BF16, kind="ExternalInput")
    rsel_d = nc.dram_tensor("rsel", [P, NT], BF16, kind="ExternalInput")
    y_d = nc.dram_tensor("y", [SC, D], BF16, kind="ExternalOutput")

    inv_sqrt_dqk = 1.0 / math.sqrt(DQK)
    qc0 = NKC - SC  # first packed col of this core's own queries (768)

    with tile.TileContext(nc) as tc:
        with (
            tc.tile_pool(name="persist", bufs=1) as persist,
            tc.tile_pool(name="ps", bufs=4, space="PSUM") as ps,
            tc.tile_pool(name="psO", bufs=2, space="PSUM") as psO,
            tc.tile_pool(name="psL", bufs=2, space="PSUM") as psL,
        ):
            # ---- persistent SBUF (lives through both phases) ----
            qT = persist.tile([P, NH, SC], BF16, tag="qT")
            kT = persist.tile([P, NKV, NKC], BF16, tag="kT")
            v_sb = persist.tile([P, NT, NKV * DV], BF16, tag="v")
            outT = persist.tile([P, NH, SC], BF16, tag="outT")
            b_sb = persist.tile([P, 11, P], BF16, tag="bm")
            rsel_sb = persist.tile([P, NT], BF16, tag="rsel")
            ones_row = persist.tile([1, P], BF16, tag="onesr")

            nc.vector.memset(ones_row[:], 1.0)

            def rope(dst, psrc, cosap, sinap, pool, w):
                """dst(bf16) = RoPE(psrc) in the paired [re(64); im(64)] basis.

                sw = [-im; re]; dst = psrc*cos + sw*sin.
                """
                sw = pool.tile([P, w], F32, tag="sw")
                nc.scalar.mul(sw[0:64], psrc[64:128], -1.0)
                nc.scalar.copy(sw[64:128], psrc[0:64])
                trc = pool.tile([P, w], F32, tag="trc")
                nc.vector.tensor_tensor(
                    trc[:], psrc, cosap, op=mybir.AluOpType.mult
                )
                nc.vector.tensor_tensor(
                    sw[:], sw[:], sinap, op=mybir.AluOpType.mult
                )
                nc.vector.tensor_tensor(
                    dst, trc[:], sw[:], op=mybir.AluOpType.add
                )

            # ---- Phase A: projections + RoPE (x/weights pool freed after) ----
            with tc.tile_pool(name="phA", bufs=1) as pa, tc.tile_pool(
                name="wqs", bufs=2
            ) as wqs, tc.tile_pool(name="ropep", bufs=3) as rp:
                x_sb = pa.tile([P, KO, NKC], BF16, tag="x")
                cos_sb = pa.tile([P, NKC], FP16, tag="cos")
                sin_sb = pa.tile([P, NKC], FP16, tag="sin")
                wk_sb = pa.tile([P, NKV, KO, DQK], BF16, tag="wk")
                wv_sb = pa.tile([P, KO, NKV * DV], BF16, tag="wv")

                nc.sync.dma_start(wk_sb[:, 0, 0:4], wk_d[:, 0, 0:4])
                nc.sync.dma_start(x_sb[:, 0:2, 0:512], x_d[:, 0:2, 0:512])
                nc.sync.dma_start(x_sb[:, 2:4, 0:512], x_d[:, 2:4, 0:512])
                nc.sync.dma_start(wk_sb[:, 0, 4:16], wk_d[:, 0, 4:16])
                for kg in range(4, KO, 4):
                    nc.sync.dma_start(
                        x_sb[:, kg : kg + 4, 0:512],
                        x_d[:, kg : kg + 4, 0:512],
                    )
                nc.sync.dma_start(cos_sb[:, 0:512], cos_d[:, 0:512])
                nc.sync.dma_start(sin_sb[:, 0:512], sin_d[:, 0:512])
                for _kvh in range(1, NKV):
                    nc.sync.dma_start(wk_sb[:, _kvh], wk_d[:, _kvh])
                nc.sync.dma_start(wv_sb[:], wv_d[:])
                for c0 in range(512, NKC, 512):
                    cw = min(512, NKC - c0)
                    nc.sync.dma_start(
                        x_sb[:, :, c0 : c0 + cw], x_d[:, :, c0 : c0 + cw]
                    )
                    nc.sync.dma_start(
                        cos_sb[:, c0 : c0 + cw], cos_d[:, c0 : c0 + cw]
                    )
                    nc.sync.dma_start(
                        sin_sb[:, c0 : c0 + cw], sin_d[:, c0 : c0 + cw]
                    )
                nc.sync.dma_start(b_sb[:], b_d[:])
                nc.sync.dma_start(rsel_sb[:], rsel_d[:])

                # k+v projection, chunk-outer so compute starts on chunk 0
                for c0 in range(0, NKC, 512):
                    cw = min(512, NKC - c0)
                    for kvh in range(NKV):
                        psk = ps.tile([P, SC], F32, tag="big", name="psk")
                        psk = psk[:, :cw]
                        for ko in range(KO):
                            nc.tensor.matmul(
                                psk,
                                wk_sb[:, kvh, ko, :],
                                x_sb[:, ko, c0 : c0 + cw],
                                start=(ko == 0),
                                stop=(ko == KO - 1),
                            )
                        rope(
                            kT[:, kvh, c0 : c0 + cw],
                            psk,
                            cos_sb[:, c0 : c0 + cw],
                            sin_sb[:, c0 : c0 + cw],
                            rp,
                            cw,
                        )
                    # v projection (natural [keys, dv], all 4 kv heads)
                    for t in range(c0 // P, min(NT, (c0 + cw) // P)):
                        psv = ps.tile([P, SC], F32, tag="big", name="psv")
                        psv = psv[:, : NKV * DV]
                        for ko in range(KO):
                            nc.tensor.matmul(
                                psv,
                                x_sb[:, ko, t * P : (t + 1) * P],
                                wv_sb[:, ko, :],
                                start=(ko == 0),
                                stop=(ko == KO - 1),
                            )
                        nc.vector.tensor_copy(v_sb[:, t, :], psv)

                # q projection + RoPE, streaming wq in 4-head groups
                for g in range(4):
                    wq_g = wqs.tile([P, KO, 4 * DQK], BF16, tag="wqg")
                    nc.sync.dma_start(wq_g[:], wq_d[:, g])
                    for hh in range(4):
                        h = 4 * g + hh
                        psq = ps.tile([P, SC], F32, tag="big", name="psq")
                        psq = psq[:, :SC]
                        for ko in range(KO):
                            nc.tensor.matmul(
                                psq,
                                wq_g[:, ko, hh * DQK : (hh + 1) * DQK],
                                x_sb[:, ko, qc0:NKC],
                                start=(ko == 0),
                                stop=(ko == KO - 1),
                            )
                        rope(
                            qT[:, h, :],
                            psq,
                            cos_sb[:, qc0:NKC],
                            sin_sb[:, qc0:NKC],
                            rp,
                            SC,
                        )

            # ---- Phase B: attention (transposed scores, per-head tile rows) --
            # Per key tile t, the q columns that can attend it:
            #   front tiles (t=0,1): all 512;  band tile b: qtl in [b-4, b].
            qr = {0: (0, SC), 1: (0, SC)}
            for b in range(8):
                lo = max(0, b - 4) * P
                hi = (min(3, b) + 1) * P
                qr[2 + b] = (lo, hi - lo)
            # phase C pools opened now so wo prefetch overlaps phase B
            pc = tc.alloc_tile_pool(name="phC", bufs=2)
            pcy = tc.alloc_tile_pool(name="phCy", bufs=4)
            with tc.tile_pool(name="phB", bufs=22) as pb, tc.tile_pool(
                name="phBs", bufs=3
            ) as pbs:
                tails = [None] * NH  # (psl, pso) pending normalize

                def tail_recip(h):
                    psl, pso = tails[h]
                    irl = pbs.tile([1, SC], F32, tag="irl", name="irl")
                    nc.vector.reciprocal_approx_fast(irl[:], psl[:])
                    return irl

                def tail_mid(h, irl):
                    lrow = pbs.tile([1, SC], BF16, tag="lrow", name="lrow")
                    nc.scalar.copy(lrow[:], irl[:])
                    return lrow

                def tail_apply(h, lrow):
                    _, pso = tails[h]
                    psbc = ps.tile([P, SC], F32, tag="big", name="psbc")
                    nc.tensor.matmul(
                        psbc[:], ones_row[:], lrow[:], start=True, stop=True
                    )
                    rlbc = pbs.tile([P, SC], BF16, tag="rlbc", name="rlbc")
                    nc.vector.tensor_copy(rlbc[:], psbc[:])
                    nc.vector.tensor_tensor(
                        outT[:, h, :], pso[:], rlbc[:], op=mybir.AluOpType.mult
                    )
                    tails[h] = None

                # masked 128-col blocks per tile: (block index, col offset)
                mask_blocks = {0: [(0, 0)], 1: [(1, 0), (2, P)]}
                for b in range(8):
                    off = (b if b < 4 else 0) * P
                    mask_blocks[2 + b] = [(3 + b, off)]

                def emit_scores(h):
                    kvh = h // NREP
                    pTs = []
                    for ti in range(NT):
                        q0, qw = qr[ti]
                        pst = ps.tile([P, SC], F32, tag="big", name="pst")
                        pst = pst[:, :qw]
                        nc.tensor.matmul(
                            pst,
                            kT[:, kvh, ti * P : (ti + 1) * P],
                            qT[:, h, q0 : q0 + qw],
                            start=True,
                            stop=True,
                        )
                        pTt = pb.tile([P, SC], BF16, tag="pT", name="pTt")
                        pTt = pTt[:, :qw]
                        nc.scalar.activation(
                            pTt,
                            pst,
                            mybir.ActivationFunctionType.Exp,
                            scale=inv_sqrt_dqk,
                        )
                        for blk, off in mask_blocks[ti]:
                            nc.vector.tensor_tensor(
                                pTt[:, off : off + P],
                                pTt[:, off : off + P],
                                b_sb[:, blk, :],
                                op=mybir.AluOpType.mult,
                            )
                        pTs.append(pTt)
                    return pTs

                def emit_la(h, pTs):
                    kvh = h // NREP
                    pso = psO.tile([P, SC], F32, tag="o", name="pso")
                    psl = psL.tile([1, SC], F32, tag="l", name="psl")
                    for ti in range(NT):
                        q0, qw = qr[ti]
                        nc.tensor.matmul(
                            psl[:, q0 : q0 + qw],
                            rsel_sb[:, ti : ti + 1],
                            pTs[ti],
                            start=(ti == 0),
                            stop=(ti == NT - 1),
                            skip_group_check=True,
                        )
                    for ti in range(NT):
                        q0, qw = qr[ti]
                        nc.tensor.matmul(
                            pso[:, q0 : q0 + qw],
                            v_sb[:, ti, kvh * DV : (kvh + 1) * DV],
                            pTs[ti],
                            start=(ti == 0),
                            stop=(ti == NT - 1),
                            skip_group_check=True,
                        )
                    tails[h] = (psl, pso)

                for hp in range(0, NH, 2):
                    lr0 = lr1 = None
                    if hp > 0:
                        lr0 = tail_mid(hp - 2, tail_recip(hp - 2))
                        lr1 = tail_mid(hp - 1, tail_recip(hp - 1))
                    pTs0 = emit_scores(hp)
                    pTs1 = emit_scores(hp + 1)
                    if hp > 0:
                        tail_apply(hp - 2, lr0)
                        tail_apply(hp - 1, lr1)
                    emit_la(hp, pTs0)
                    emit_la(hp + 1, pTs1)
                tail_apply(NH - 2, tail_mid(NH - 2, tail_recip(NH - 2)))
                tail_apply(NH - 1, tail_mid(NH - 1, tail_recip(NH - 1)))

            # ---- Phase C: y = outT^T @ wo (stream wo in n-chunks) ----
            y_tiles = [
                pcy.tile([P, D], BF16, tag="y", name=f"y{i}")
                for i in range(NQTL)
            ]
            for ncl in range(4):
                wo_g = pc.tile([P, NH, SC], BF16, tag="wog", name="wo_g")
                nc.sync.dma_start(wo_g[:], wo_d[:, ncl])
                for qtl in range(NQTL):
                    psy = ps.tile([P, SC], F32, tag="big", name="psy")
                    for h in range(NH):
                        nc.tensor.matmul(
                            psy[:],
                            outT[:, h, qtl * P : (qtl + 1) * P],
                            wo_g[:, h, :],
                            start=(h == 0),
                            stop=(h == NH - 1),
                        )
                    nc.vector.tensor_copy(
                        y_tiles[qtl][:, ncl * SC : (ncl + 1) * SC], psy[:]
                    )
                    nc.sync.dma_start(
                        y_d[
                            qtl * P : (qtl + 1) * P,
                            ncl * SC : (ncl + 1) * SC,
                        ],
                        y_tiles[qtl][:, ncl * SC : (ncl + 1) * SC],
                    )
            pcy.release()
            pc.release()

    return nc


_PROGRAM = None


def _get_program():
    global _PROGRAM
    if _PROGRAM is None:
        _PROGRAM = build_program()
        _PROGRAM.finalize()
    return _PROGRAM


def _host_inputs(x, wq, wk, wv, wo):
    """Per-core input packing (all arrays contiguous, uniform shapes)."""
    x2 = np.asarray(x, np.float32).reshape(S, D)
    xT = np.ascontiguousarray(x2.T)  # [D, S]
    xr = xT.reshape(KO, P, S)  # [ko, p, s]

    # paired RoPE basis permutation within each head
    perm = np.concatenate([np.arange(0, DQK, 2), np.arange(1, DQK, 2)])
    wq_p = np.asarray(wq, np.float32).reshape(D, NH, DQK)[:, :, perm]
    wk_p = np.asarray(wk, np.float32).reshape(D, NKV, DQK)[:, :, perm]
    wv_r = np.asarray(wv, np.float32).reshape(D, NKV * DV)
    wo_r = np.asarray(wo, np.float32).reshape(NH, DV, D)

    # device layouts independent of core
    wq_dev = np.ascontiguousarray(
        wq_p.reshape(KO, P, NH, DQK)  # [ko, p, h, dqk]
        .reshape(KO, P, 4, 4 * DQK)  # group 4 heads
        .transpose(1, 2, 0, 3)  # [p, g, ko, 4*dqk]
    ).astype(ml_bf16)
    wk_dev = np.ascontiguousarray(
        wk_p.reshape(KO, P, NKV, DQK).transpose(1, 2, 0, 3)
    ).astype(ml_bf16)
    wv_dev = np.ascontiguousarray(
        wv_r.reshape(KO, P, NKV * DV).transpose(1, 0, 2)
    ).astype(ml_bf16)
    wo_dev = np.ascontiguousarray(
        wo_r.reshape(NH, DV, 4, SC).transpose(1, 2, 0, 3)  # [dv, ncl, h, sc]
    ).astype(ml_bf16)

    inv_freq = 1.0 / (THETA ** (np.arange(0, DQK, 2)[: DQK // 2] / DQK))

    in_maps = []
    for c in range(NC_):
        qlo = c * SC
        band_lo = qlo - WIN
        # packed key positions; garbage (pos<0) -> position 0, zero x
        pos = np.empty(NKC, np.int64)
        pos[: FRONT] = np.arange(FRONT)
        pos[FRONT:] = band_lo + np.arange(NKC - FRONT)
        valid = pos >= 0
        pos_c = np.where(valid, pos, 0)

        xp = xr[:, :, pos_c] * valid[None, None, :]  # [ko, p, nkc]
        if c == 0:
            # front tiles are dead on core 0 (band covers them); zero x so
            # their v projection is 0 and unmasked pT blocks are harmless
            xp[:, :, :FRONT] = 0.0
        xp = np.ascontiguousarray(xp.transpose(1, 0, 2)).astype(ml_bf16)

        ang = np.outer(pos_c.astype(np.float64), inv_freq)  # (nkc, 64)
        cos_h = np.cos(ang).T.astype(np.float32)  # (64, nkc)
        sin_h = np.sin(ang).T.astype(np.float32)
        cos_p = np.ascontiguousarray(
            np.concatenate([cos_h, cos_h], 0)
        ).astype(np.float16)
        sin_p = np.ascontiguousarray(
            np.concatenate([sin_h, sin_h], 0)
        ).astype(np.float16)

        # Sparse 128x128 mask blocks (only blocks non-full on some core):
        #   blk 0: (front0, qtl0)  blk 1: (front1, qtl0)  blk 2: (front1, qtl1)
        #   blk 3+b: (band b, qtl b) for b<4 else (band b, qtl b-4)
        # Front tiles use only front_ok, band tiles only the band condition.
        r = np.arange(P)[:, None]
        B = np.zeros((P, 11, P), np.float32)
        blocks = [(0, 0), (1, 0), (1, 1)] + [
            (2 + b, b if b < 4 else b - 4) for b in range(8)
        ]
        for blk, (t, qtl) in enumerate(blocks):
            qpos = qlo + qtl * P + np.arange(P)[None, :]
            if t < 2:
                kpos = t * P + r
                allowed = (kpos < FRONT) & (kpos <= qpos - WIN)
            else:
                kpos = band_lo + (t - 2) * P + r
                allowed = (kpos >= 0) & (kpos <= qpos) & (kpos > qpos - WIN)
            B[:, blk, :] = allowed
        Bp = np.ascontiguousarray(B).astype(ml_bf16)
        # row-select for the lsum reduction: excludes padding rows and,
        # on cores 0-1 ... front rows are excluded only when front tiles
        # are dead (core 0, whose front x is zeroed; band covers front).
        rsel = np.zeros((P, NT), np.float32)
        for t in range(NT):
            if t < 2:
                rsel[:, t] = 0.0 if c == 0 else 1.0
            else:
                kpos = band_lo + (t - 2) * P + r[:, 0]
                rsel[:, t] = (kpos >= 0).astype(np.float32)
        rsel_p = np.ascontiguousarray(rsel).astype(ml_bf16)

        in_maps.append(
            {
                "xp": xp,
                "wq": wq_dev,
                "wk": wk_dev,
                "wv": wv_dev,
                "wo": wo_dev,
                "cosd": cos_p,
                "sind": sin_p,
                "bmask": Bp,
                "rsel": rsel_p,
            }
        )
    return in_maps


def kernel(x, wq, wk, wv, wo, _trace=False, _trace_kwargs=None):
    nc = _get_program()
    in_maps = _host_inputs(x, wq, wk, wv, wo)
    res = run_bass_kernel_spmd(
        nc, in_maps, list(range(NC_)), trace=_trace, **(_trace_kwargs or {})
    )
    y = np.concatenate(
        [np.asarray(r["y"], np.float32) for r in res.results], axis=0
    )
    out = y.reshape(1, S, D)
    if _trace:
        return out, res
    return out
